# revision 24
# baseline (speedup 1.0000x reference)
"""Trainium2 Bass kernel: 16-head MultiHeadAttention (B=2, S=2048, D=1024, F=64).

Sharding: 8 cores = 2 batches x 4 sequence-quarters. Each core handles 512
query rows for ALL heads of one batch (needs full K/V of that batch; K/V
projections are recomputed per quarter to avoid on-device collectives, which
run far below link rate and would dominate).

Per-core dataflow (all layouts chosen so no on-chip transposes are needed):
  qT[f,s]   = (Wq~.T @ Xq.T)         pair-packed over heads, scale 1/8 folded in
  kT[f,t]   = (Wk~.T @ Xk.T)
  v[t,hf]   = (Xv.T.T @ Wv~)         + ones column per head (rowsum trick)
  sT[t,s]   = kT.T @ qT              per head, then exp() only (no max shift)
  U[f',s]   = v_aug.T @ exp(sT)      rows 0-63 = (A_unnorm @ V).T, row 64 = rowsum
  hcT[hf,s] = U[0:64]/rowsum + bv    1/rowsum broadcast on GpSimd, DVE mult/add
  out[s,d]  = hcT.T @ Wo.T + bo      full contraction over heads -> disjoint rows

Phase structure interleaves Q/K projections of pair p+1 with attention of
pair p so the ScalarE exp stream (the per-pair critical resource) starts as
early as possible and hides under TensorE work.
"""

import numpy as np
import ml_dtypes

import concourse.bass as bass
import concourse.bacc as bacc_mod
import concourse.tile as tile
from concourse import mybir
from concourse.bass_utils import run_bass_kernel_spmd

D, FH, H, B, S = 1024, 64, 16, 2, 2048
SQ = S // 4  # query rows per core
NCORES = 8
BF16 = mybir.dt.bfloat16
F32 = mybir.dt.float32
nbf16 = ml_dtypes.bfloat16

_cache = {}

# module-global: last BassKernelResults (for profiling from test harness)
last_results = None


def build_program():
    nc = bacc_mod.Bacc(None, target_bir_lowering=False)
    xqT = nc.dram_tensor("xqT", [D, SQ], BF16, kind="ExternalInput")
    xkT = nc.dram_tensor("xkT", [D, S], BF16, kind="ExternalInput")
    xvT = nc.dram_tensor("xvT", [D, S], BF16, kind="ExternalInput")
    # weights host-packed for per-partition-contiguous DMA:
    # wq/wk: [q, pair, a, n] with d = a*128+q, col = pair*128+n
    wq = nc.dram_tensor("wq", [128, 8, 8, 128], BF16, kind="ExternalInput")
    wk = nc.dram_tensor("wk", [128, 8, 8, 128], BF16, kind="ExternalInput")
    wv = nc.dram_tensor("wv", [D, D], BF16, kind="ExternalInput")
    # woT: [q, pair, n] with hf = pair*128+q
    woT = nc.dram_tensor("woT", [128, 8, D], BF16, kind="ExternalInput")
    bq_d = nc.dram_tensor("bq", [D], F32, kind="ExternalInput")
    bk_d = nc.dram_tensor("bk", [D], F32, kind="ExternalInput")
    bv_d = nc.dram_tensor("bv", [D], F32, kind="ExternalInput")
    bo_d = nc.dram_tensor("bo", [D], F32, kind="ExternalInput")
    out_d = nc.dram_tensor("out", [SQ, D], F32, kind="ExternalOutput")

    Exp = mybir.ActivationFunctionType.Exp

    with tile.TileContext(nc) as tc:
        with tc.tile_pool(name="persist", bufs=1) as persist:
            qT = persist.tile([128, 8, SQ], BF16)     # [r=(h%2)*64+f, pair, s]
            kT = persist.tile([128, 8, S], BF16)      # [r, pair, t]
            vsb = persist.tile([128, 16, 16 * 65], BF16)  # [t%128, t//128, h*65+f']
            hcT = persist.tile([128, 8, SQ], BF16)    # [r, pair, s]
            bq_t = persist.tile([128, 8], F32)
            bk_t = persist.tile([128, 8], F32)
            bv_pp = persist.tile([128, 8], F32)
            bob = persist.tile([128, D], F32)
            ones64 = persist.tile([1, 64], F32)
            nc.vector.memset(ones64, 1.0)

            with tc.tile_pool(name="xqk", bufs=1) as xqk, \
                 tc.tile_pool(name="ws", bufs=2) as ws, \
                 tc.tile_pool(name="pj", bufs=2, space="PSUM") as pj:
                xq_t = xqk.tile([128, 8, SQ], BF16)
                xk_t = xqk.tile([128, 8, S], BF16)
                for d in range(8):
                    nc.sync.dma_start(out=xq_t[:, d, :], in_=xqT[d * 128:(d + 1) * 128, :])

                def qk_proj(p):
                    wqt = ws.tile([128, 8, 128], BF16, name="wqt", tag="wq")
                    wkt = ws.tile([128, 8, 128], BF16, name="wkt", tag="wk")
                    nc.sync.dma_start(out=wqt, in_=wq[:, p, :, :])
                    nc.sync.dma_start(out=wkt, in_=wk[:, p, :, :])
                    if p == 0:
                        # per-(d, chunk) loads so the first K chain starts on
                        # the first 2MB instead of the full 8MB
                        for c in range(4):
                            for d in range(8):
                                nc.sync.dma_start(out=xk_t[:, d, c * 512:(c + 1) * 512],
                                                  in_=xkT[d * 128:(d + 1) * 128, c * 512:(c + 1) * 512])
                    qps = pj.tile([128, SQ], F32, name="qps", tag="qk")
                    for d in range(8):
                        nc.tensor.matmul(qps, wqt[:, d, :], xq_t[:, d, :], start=(d == 0), stop=(d == 7))
                    nc.vector.tensor_scalar_add(qT[:, p, :], qps, bq_t[:, p:p + 1])
                    for c in range(4):
                        kps = pj.tile([128, 512], F32, name="kps", tag="qk")
                        for d in range(8):
                            nc.tensor.matmul(kps, wkt[:, d, :], xk_t[:, d, c * 512:(c + 1) * 512],
                                             start=(d == 0), stop=(d == 7))
                        nc.vector.tensor_scalar_add(kT[:, p, c * 512:(c + 1) * 512], kps, bk_t[:, p:p + 1])

                # biases via SWDGE (GpSimd) so they don't delay the SP HWDGE queue
                nc.gpsimd.dma_start(out=bq_t, in_=bq_d[:].rearrange("(a p) -> p a", p=128))
                nc.gpsimd.dma_start(out=bk_t, in_=bk_d[:].rearrange("(a p) -> p a", p=128))
                nc.gpsimd.dma_start(out=bv_pp, in_=bv_d[:].rearrange("(a p) -> p a", p=128))
                nc.gpsimd.dma_start(out=bob, in_=bo_d[:].partition_broadcast(128))

                # pre-touch DMA-loaded tiles on DVE: DVE instruction structs
                # hold only ONE embedded sync wait, so real consumers (which
                # also wait on PE) must not additionally wait on the DMA queue.
                dve_touch = persist.tile([1, 4], F32)
                nc.vector.tensor_copy(dve_touch[:, 0:1], bob[0:1, 0:1])
                nc.vector.tensor_copy(dve_touch[:, 1:2], bq_t[0:1, 0:1])
                nc.vector.tensor_copy(dve_touch[:, 2:3], bk_t[0:1, 0:1])
                nc.vector.tensor_copy(dve_touch[:, 3:4], bv_pp[0:1, 0:1])

                vsb4 = vsb.rearrange("p t (h f) -> p t h f", f=65)
                nc.vector.memset(vsb4[:, :, :, 64:65], 1.0)

                qk_proj(0)

                for p_early in (1, 2, 3):
                    qk_proj(p_early)

                # V projection, t-major
                with tc.tile_pool(name="xvw", bufs=1) as xvw, \
                     tc.tile_pool(name="vp", bufs=2, space="PSUM") as vp:
                    xv_t = xvw.tile([128, 8, S], BF16)
                    wv_t = xvw.tile([128, 8, D], BF16)
                    for d in range(8):
                        nc.sync.dma_start(out=xv_t[:, d, :], in_=xvT[d * 128:(d + 1) * 128, :])
                        nc.sync.dma_start(out=wv_t[:, d, :], in_=wv[d * 128:(d + 1) * 128, :])
                    for t in range(16):
                        vps = vp.tile([128, 1024], F32, name="vps", tag="v")
                        for c in range(2):
                            for d in range(8):
                                nc.tensor.matmul(vps[:, c * 512:(c + 1) * 512],
                                                 xv_t[:, d, t * 128:(t + 1) * 128],
                                                 wv_t[:, d, c * 512:(c + 1) * 512],
                                                 start=(d == 0), stop=(d == 7))
                        vps3 = vps.rearrange("p (h f) -> p h f", f=64)
                        nc.vector.tensor_copy(vsb4[:, t, :, 0:64], vps3)

                # attention, pair-interleaved with next pair's projections
                with tc.tile_pool(name="rsd", bufs=2, space="DRAM") as rsdp, \
                     tc.tile_pool(name="expp", bufs=8) as expp, \
                     tc.tile_pool(name="rsp", bufs=2) as rsp, \
                     tc.tile_pool(name="wop", bufs=1) as wop, \
                     tc.tile_pool(name="outp", bufs=4) as outp:
                    wo_t = wop.tile([128, 8, D], BF16)
                    nc.sync.dma_start(out=wo_t, in_=woT[:])

                    with tc.tile_pool(name="sc", bufs=1, space="PSUM") as scp, \
                         tc.tile_pool(name="up", bufs=2, space="PSUM") as up:
                        for p in range(8):
                            # the two heads of a pair sit in PE row-groups
                            # 0-63 / 64-127; adjacent emission lets their K=64
                            # score matmuls execute concurrently in the array
                            ets = [[], []]
                            for g in range(8):
                                sc_pair = []
                                for r_ in range(2):
                                    erow = slice(64 * r_, 64 * (r_ + 1))
                                    scps = sc_pair_t = scp.tile([128, 1024], F32, name=f"scps{r_}", tag=f"sc{r_}")
                                    for half in range(2):
                                        j = 2 * g + half
                                        nc.tensor.matmul(scps[:, half * 512:(half + 1) * 512],
                                                         kT[erow, p, j * 128:(j + 1) * 128],
                                                         qT[erow, p, :],
                                                         start=True, stop=True)
                                    sc_pair.append(scps)
                                for r_ in range(2):
                                    et = expp.tile([128, 1024], BF16, name=f"et{r_}", tag=f"exp{r_}")
                                    nc.scalar.activation(et, sc_pair[r_], Exp)
                                    ets[r_].append(et)
                            Us = []
                            for r_ in range(2):
                                h = 2 * p + r_
                                U = up.tile([65, 512], F32, name=f"U{r_}", tag="U")
                                for j in range(16):
                                    nc.tensor.matmul(U, vsb4[:, j, h, :],
                                                     ets[r_][j // 2][:, (j % 2) * 512:(j % 2 + 1) * 512],
                                                     start=(j == 0), stop=(j == 15))
                                Us.append(U)
                            rc0 = rsp.tile([1, 512], F32, name="rc0", tag="rc0")
                            rc1 = rsp.tile([1, 512], F32, name="rc1", tag="rc1")
                            nc.vector.reciprocal(rc0, Us[0][64:65, :])
                            nc.vector.reciprocal(rc1, Us[1][64:65, :])
                            # broadcast 1/rowsum via DRAM bounce: DMA is the
                            # only engine that can fan one partition out to many
                            rsd = rsdp.tile([2, 512], F32, name="rsd", tag="rsd")
                            nc.sync.dma_start(out=rsd[0:1, :], in_=rc0)
                            nc.sync.dma_start(out=rsd[1:2, :], in_=rc1)
                            rbsb = rsp.tile([128, 512], F32, name="rbsb", tag="rbsb")
                            nc.sync.dma_start(out=rbsb[0:64, :],
                                              in_=rsd[0:1, :].rearrange("a n -> (a n)").partition_broadcast(64))
                            nc.sync.dma_start(out=rbsb[64:128, :],
                                              in_=rsd[1:2, :].rearrange("a n -> (a n)").partition_broadcast(64))
                            # DVE pre-touch absorbs the DMA wait (1-wait limit)
                            nc.vector.tensor_copy(dve_touch[:, 0:1], rbsb[0:1, 0:1])
                            nc.vector.tensor_copy(dve_touch[:, 1:2], rbsb[64:65, 0:1])
                            nc.vector.tensor_mul(hcT[0:64, p, :], Us[0][0:64, :], rbsb[0:64, :])
                            nc.vector.tensor_mul(hcT[64:128, p, :], Us[1][0:64, :], rbsb[64:128, :])
                            nc.vector.tensor_scalar_add(hcT[:, p, :], hcT[:, p, :], bv_pp[:, p:p + 1])
                            if p < 4:
                                qk_proj(p + 4)

                    # output projection: fp pool reuses sc/up banks (pj still
                    # holds 2 banks, so 4 tags x 1 buf = 4 banks fits)
                    with tc.tile_pool(name="fp", bufs=1, space="PSUM") as fpp:
                        for dc in range(2):
                            fps = [fpp.tile([128, 512], F32, name=f"fps{si}", tag=f"f{si}") for si in range(4)]
                            for p in range(8):
                                for si in range(4):
                                    nc.tensor.matmul(fps[si],
                                                     hcT[:, p, si * 128:(si + 1) * 128],
                                                     wo_t[:, p, dc * 512:(dc + 1) * 512],
                                                     start=(p == 0), stop=(p == 7))
                            for si in range(4):
                                ot = outp.tile([128, 512], F32, name="ot", tag="ot")
                                nc.vector.tensor_add(ot, fps[si], bob[:, dc * 512:(dc + 1) * 512])
                                nc.sync.dma_start(out=out_d[si * 128:(si + 1) * 128, dc * 512:(dc + 1) * 512], in_=ot)

    nc.compile()
    return nc


def make_in_maps(inputs):
    q = np.asarray(inputs["queries"], np.float32)
    k = np.asarray(inputs["keys"], np.float32)
    v = np.asarray(inputs["values"], np.float32)
    Wq = np.asarray(inputs["Wq"], np.float32)
    Wk = np.asarray(inputs["Wk"], np.float32)
    Wv = np.asarray(inputs["Wv"], np.float32)
    bq = np.asarray(inputs["bq"], np.float32)
    bk = np.asarray(inputs["bk"], np.float32)
    bv = np.asarray(inputs["bv"], np.float32)
    Wo = np.asarray(inputs["Wo"], np.float32)
    bo = np.asarray(inputs["bo"], np.float32)

    scale = 1.0 / np.sqrt(np.float32(FH))
    wq_pk = (Wq.transpose(1, 0, 2).reshape(D, D) * scale).astype(nbf16)
    wk_pk = Wk.transpose(1, 0, 2).reshape(D, D).astype(nbf16)
    # [q, pair, a, n] packing for per-partition-contiguous pair slices
    wq_h = np.ascontiguousarray(wq_pk.reshape(8, 128, 8, 128).transpose(1, 2, 0, 3))
    wk_h = np.ascontiguousarray(wk_pk.reshape(8, 128, 8, 128).transpose(1, 2, 0, 3))
    wv_pk = np.ascontiguousarray(Wv.transpose(1, 0, 2).reshape(D, D)).astype(nbf16)
    woT_bf = Wo.T.astype(nbf16)
    wo_h = np.ascontiguousarray(woT_bf.reshape(8, 128, D).transpose(1, 0, 2))
    bq_pk = (bq.reshape(-1) * scale).astype(np.float32)
    bk_pk = bk.reshape(-1).astype(np.float32)
    bv_pk = bv.reshape(-1).astype(np.float32)
    bo_pk = bo.reshape(-1).astype(np.float32)

    kTb = [np.ascontiguousarray(k[b].T).astype(nbf16) for b in range(B)]
    vTb = [np.ascontiguousarray(v[b].T).astype(nbf16) for b in range(B)]
    qTb = [np.ascontiguousarray(q[b].T).astype(nbf16) for b in range(B)]

    in_maps = []
    for c in range(NCORES):
        b, qc = divmod(c, 4)
        in_maps.append({
            "xqT": np.ascontiguousarray(qTb[b][:, qc * SQ:(qc + 1) * SQ]),
            "xkT": kTb[b],
            "xvT": vTb[b],
            "wq": wq_h, "wk": wk_h, "wv": wv_pk, "woT": wo_h,
            "bq": bq_pk, "bk": bk_pk, "bv": bv_pk, "bo": bo_pk,
        })
    return in_maps


def kernel(**inputs):
    global last_results
    import os
    if "nc" not in _cache:
        _cache["nc"] = build_program()
    nc = _cache["nc"]
    in_maps = make_in_maps(inputs)
    trace = os.environ.get("KERNEL_TRACE", "0") == "1"
    res = run_bass_kernel_spmd(nc, in_maps, list(range(NCORES)), trace=trace)
    last_results = res
    out = np.empty((B, S, D), np.float32)
    for c in range(NCORES):
        b, qc = divmod(c, 4)
        out[b, qc * SQ:(qc + 1) * SQ, :] = res.results[c]["out"]
    return out
